# revision 6
# baseline (speedup 1.0000x reference)
"""Trainium2 Bass kernel for a single-step LSTM cell (nn_BasicLSTM).

reference math (B=32768, IN=H=512, NCLS=2):
    f = sigmoid(x @ U_f + h @ V_f + b_f)      i = sigmoid(x @ U_i + h @ V_i + b_i)
    o = sigmoid(x @ U_o + h @ V_o + b_o)      c_hat = tanh(x @ U_c + h @ V_c + b_c)
    c_new = f * c + i * c_hat                 h_new = o * tanh(c_new)
    pre_label = softmax(h_new @ W_out + b_out, axis=1)   # 2 classes

Strategy: pure data parallel over batch across 8 NeuronCores (4096 rows each).
Per core, 8 supertiles of 512 batch rows (each = 4 matmul subtiles of 128):
  - SWDGE cast-DMA x/h supertile f32 -> bf16 DRAM scratch, then one xbar
    DMA-transpose load per tensor into [K=feature, M=batch] chunk layout
    (SBUF->SBUF xbar transpose is fatal on this HW -- bounce via DRAM)
  - c supertile SWDGE cast-loaded straight to SBUF bf16
  - 4 gates = 4 PSUM 512-slices per 128-row subtile; per gate 1 bias matmul
    (ones-row trick) + 8 accumulating K=128 bf16 matmuls
  - ACT applies sigmoid/tanh straight out of PSUM into bf16 SBUF
  - DVE elementwise tail on whole supertiles; 2-class softmax folded to
    sigmoid(+/-d), d = h_new . (W_out[:,0]-W_out[:,1]) + (b_out[0]-b_out[1])
    via tensor_tensor_reduce
  - SWDGE cast-stores bf16 -> f32
"""
import sys

for _p in ("/root/.axon_site", "/root/.axon_site/_ro/trn_rl_repo",
           "/root/.axon_site/_ro/pypackages", "/opt/trn_rl_repo"):
    if _p not in sys.path:
        sys.path.append(_p)

import numpy as np
import ml_dtypes

import concourse.bacc as bacc
import concourse.tile as tile
from concourse import mybir
from concourse.bass_utils import run_bass_kernel_spmd

P = 128
H = 512
KC = H // P          # 4 feature chunks of 128
NCORES = 8
B = 32768
B_LOCAL = B // NCORES
NGATES = 4           # f, i, o, c_hat
GW = NGATES * H      # 2048 = concatenated gate width
ST = 512             # supertile batch rows
SUB = ST // P        # 4 subtiles per supertile

BF16 = mybir.dt.bfloat16
F32 = mybir.dt.float32
Sigmoid = mybir.ActivationFunctionType.Sigmoid
Tanh = mybir.ActivationFunctionType.Tanh


def build_program(b_local=B_LOCAL, num_devices=NCORES):
    nst = b_local // ST
    nc = bacc.Bacc("TRN2", target_bir_lowering=False, debug=False,
                   num_devices=num_devices)

    x_d = nc.dram_tensor("input_vector", (b_local, H), F32, kind="ExternalInput").ap()
    h_d = nc.dram_tensor("hidden_state", (b_local, H), F32, kind="ExternalInput").ap()
    c_d = nc.dram_tensor("cell_state", (b_local, H), F32, kind="ExternalInput").ap()
    # Wx[p, k, g*512+n] = U_g[k*128+p, n]; same for Wh/V_g. bf16, host-arranged.
    wx_d = nc.dram_tensor("Wx", (P, KC, GW), BF16, kind="ExternalInput").ap()
    wh_d = nc.dram_tensor("Wh", (P, KC, GW), BF16, kind="ExternalInput").ap()
    bias_d = nc.dram_tensor("bias_cat", (1, GW), BF16, kind="ExternalInput").ap()
    wd_d = nc.dram_tensor("w_diff", (1, H), BF16, kind="ExternalInput").ap()
    bd_d = nc.dram_tensor("b_diff", (1, 2), F32, kind="ExternalInput").ap()  # [+bd, -bd]

    pl_d = nc.dram_tensor("pre_label", (b_local, 2), F32, kind="ExternalOutput").ap()
    hn_d = nc.dram_tensor("h_new", (b_local, H), F32, kind="ExternalOutput").ap()
    cn_d = nc.dram_tensor("c_new", (b_local, H), F32, kind="ExternalOutput").ap()

    with tile.TileContext(nc) as tc:
        with tc.tile_pool(name="const", bufs=1) as const, \
             tc.tile_pool(name="work", bufs=2) as work, \
             tc.tile_pool(name="scr", bufs=3, space="DRAM") as scr, \
             tc.tile_pool(name="psum", bufs=2, space="PSUM") as psp:

            wx_sb = const.tile([P, KC, GW], BF16)
            nc.sync.dma_start(out=wx_sb, in_=wx_d)
            wh_sb = const.tile([P, KC, GW], BF16)
            nc.sync.dma_start(out=wh_sb, in_=wh_d)
            bias_sb = const.tile([1, GW], BF16)
            nc.sync.dma_start(out=bias_sb, in_=bias_d)
            ones = const.tile([1, P], BF16)
            nc.vector.memset(ones, 1.0)
            wd_sb = const.tile([P, H], BF16)
            nc.gpsimd.dma_start(out=wd_sb, in_=wd_d.to_broadcast((P, H)))
            bd_sb = const.tile([P, 2], F32)
            nc.gpsimd.dma_start(out=bd_sb, in_=bd_d.to_broadcast((P, 2)))
            p_all = const.tile([P, b_local // P, 2], F32)

            for st in range(nst):
                rows = slice(st * ST, (st + 1) * ST)

                x_scr = scr.tile([ST, H], BF16, tag="x_scr")
                nc.gpsimd.dma_start(out=x_scr, in_=x_d[rows, :])
                h_scr = scr.tile([ST, H], BF16, tag="h_scr")
                nc.gpsimd.dma_start(out=h_scr, in_=h_d[rows, :])

                xT = work.tile([P, KC, ST], BF16, tag="xT")
                nc.sync.dma_start(out=xT, in_=x_scr, transpose=True)
                hT = work.tile([P, KC, ST], BF16, tag="hT")
                nc.sync.dma_start(out=hT, in_=h_scr, transpose=True)

                c_bf = work.tile([P, SUB, H], BF16, tag="c_bf")
                nc.gpsimd.dma_start(
                    out=c_bf, in_=c_d[rows, :].rearrange("(s p) f -> p s f", p=P))

                gf = work.tile([P, SUB, H], BF16, tag="gf")
                gi = work.tile([P, SUB, H], BF16, tag="gi")
                go = work.tile([P, SUB, H], BF16, tag="go")
                gc = work.tile([P, SUB, H], BF16, tag="gc")
                gtiles = (gf, gi, go, gc)

                for s in range(SUB):
                    bs = slice(s * P, (s + 1) * P)
                    psum = psp.tile([P, GW], F32, tag="psum")
                    for g in range(NGATES):
                        gs = slice(g * H, (g + 1) * H)
                        nc.tensor.matmul(psum[:, gs], ones, bias_sb[:, gs],
                                         start=True, stop=False)
                    for k in range(KC):
                        for g in range(NGATES):
                            gs = slice(g * H, (g + 1) * H)
                            nc.tensor.matmul(psum[:, gs], xT[:, k, bs],
                                             wx_sb[:, k, gs],
                                             start=False, stop=False)
                    for k in range(KC):
                        for g in range(NGATES):
                            gs = slice(g * H, (g + 1) * H)
                            nc.tensor.matmul(psum[:, gs], hT[:, k, bs],
                                             wh_sb[:, k, gs],
                                             start=False, stop=(k == KC - 1))
                    for g in range(NGATES):
                        gs = slice(g * H, (g + 1) * H)
                        nc.scalar.activation(gtiles[g][:, s, :], psum[:, gs],
                                             Tanh if g == 3 else Sigmoid)

                flat = lambda t: t.rearrange("p s f -> p (s f)")
                t1 = work.tile([P, SUB * H], BF16, tag="t1")
                nc.vector.tensor_tensor(t1, flat(gf), flat(c_bf),
                                        mybir.AluOpType.mult)
                t2 = work.tile([P, SUB * H], BF16, tag="t2")
                nc.vector.tensor_tensor(t2, flat(gi), flat(gc),
                                        mybir.AluOpType.mult)
                cn = work.tile([P, SUB, H], BF16, tag="cn")
                nc.vector.tensor_tensor(flat(cn), t1, t2, mybir.AluOpType.add)
                tt = work.tile([P, SUB * H], BF16, tag="tt")
                nc.scalar.activation(tt, flat(cn), Tanh)
                hn = work.tile([P, SUB, H], BF16, tag="hn")
                nc.vector.tensor_tensor(flat(hn), flat(go), tt,
                                        mybir.AluOpType.mult)

                nc.gpsimd.dma_start(
                    out=cn_d[rows, :].rearrange("(s p) f -> p s f", p=P), in_=cn)
                nc.gpsimd.dma_start(
                    out=hn_d[rows, :].rearrange("(s p) f -> p s f", p=P), in_=hn)

                for s in range(SUB):
                    prod = work.tile([P, H], BF16, tag="prod")
                    nc.vector.tensor_tensor(prod, hn[:, s, :], wd_sb,
                                            mybir.AluOpType.mult)
                    d_sb = work.tile([P, 1], F32, tag="d_sb")
                    nc.vector.tensor_reduce(d_sb, prod, axis=mybir.AxisListType.X,
                                            op=mybir.AluOpType.add)
                    t_idx = st * SUB + s
                    nc.scalar.activation(p_all[:, t_idx, 0:1], d_sb, Sigmoid,
                                         bias=bd_sb[:, 0:1])
                    nc.scalar.activation(p_all[:, t_idx, 1:2], d_sb, Sigmoid,
                                         bias=bd_sb[:, 1:2], scale=-1.0)

            nc.sync.dma_start(
                out=pl_d.rearrange("(t p) c -> p t c", p=P), in_=p_all)

    nc.compile()
    return nc


_CACHE = {}


def _get_program():
    key = (B_LOCAL, NCORES)
    if key not in _CACHE:
        _CACHE[key] = build_program()
    return _CACHE[key]


def _prep_weights(U_f, V_f, b_f, U_i, V_i, b_i, U_o, V_o, b_o, U_c, V_c, b_c,
                  W_out, b_out):
    bf = ml_dtypes.bfloat16
    # [128, 4, 2048]: Wx[p, k, g*512:+512] = U_g[k*128+p, :]
    def arrange(*ms):
        cat = np.concatenate([np.asarray(m, np.float32).reshape(KC, P, H)
                              .transpose(1, 0, 2)[:, :, None, :] for m in ms], axis=2)
        return np.ascontiguousarray(cat.reshape(P, KC, GW)).astype(bf)

    wx = arrange(U_f, U_i, U_o, U_c)
    wh = arrange(V_f, V_i, V_o, V_c)
    bias = np.concatenate([np.asarray(b, np.float32) for b in (b_f, b_i, b_o, b_c)]
                          ).reshape(1, GW).astype(bf)
    W_out = np.asarray(W_out, np.float32)
    b_out = np.asarray(b_out, np.float32)
    wd = (W_out[:, 0] - W_out[:, 1]).reshape(1, H).astype(bf)
    bdv = float(b_out[0]) - float(b_out[1])
    bd = np.array([[bdv, -bdv]], np.float32)
    return wx, wh, bias, wd, bd


def make_in_maps(inputs):
    wx, wh, bias, wd, bd = _prep_weights(
        inputs["U_f"], inputs["V_f"], inputs["b_f"],
        inputs["U_i"], inputs["V_i"], inputs["b_i"],
        inputs["U_o"], inputs["V_o"], inputs["b_o"],
        inputs["U_c"], inputs["V_c"], inputs["b_c"],
        inputs["W_out"], inputs["b_out"])
    x = np.ascontiguousarray(np.asarray(inputs["input_vector"], np.float32))
    h = np.ascontiguousarray(np.asarray(inputs["hidden_state"], np.float32))
    c = np.ascontiguousarray(np.asarray(inputs["cell_state"], np.float32))
    in_maps = []
    for i in range(NCORES):
        rows = slice(i * B_LOCAL, (i + 1) * B_LOCAL)
        in_maps.append({
            "input_vector": x[rows], "hidden_state": h[rows],
            "cell_state": c[rows],
            "Wx": wx, "Wh": wh, "bias_cat": bias, "w_diff": wd, "b_diff": bd,
        })
    return in_maps


def kernel(**inputs):
    nc = _get_program()
    in_maps = make_in_maps(inputs)
    res = run_bass_kernel_spmd(nc, in_maps, core_ids=list(range(NCORES)))
    pre_label = np.concatenate([r["pre_label"] for r in res.results], axis=0)
    h_new = np.concatenate([r["h_new"] for r in res.results], axis=0)
    c_new = np.concatenate([r["c_new"] for r in res.results], axis=0)
    return pre_label, h_new, c_new


# revision 8
# speedup vs baseline: 206.3468x; 206.3468x over previous
"""Trainium2 Bass kernel for a single-step LSTM cell (nn_BasicLSTM).

reference math (B=32768, IN=H=512, NCLS=2):
    f = sigmoid(x @ U_f + h @ V_f + b_f)      i = sigmoid(x @ U_i + h @ V_i + b_i)
    o = sigmoid(x @ U_o + h @ V_o + b_o)      c_hat = tanh(x @ U_c + h @ V_c + b_c)
    c_new = f * c + i * c_hat                 h_new = o * tanh(c_new)
    pre_label = softmax(h_new @ W_out + b_out, axis=1)   # 2 classes

Strategy: pure data parallel over batch across 8 NeuronCores (4096 rows each).
Per core, 8 supertiles of 512 batch rows (each = 4 matmul subtiles of 128):
  - SWDGE cast-DMA x/h supertile f32 -> bf16 DRAM scratch, then one xbar
    DMA-transpose load per tensor into [K=feature, M=batch] chunk layout
    (SBUF->SBUF xbar transpose is fatal on this HW -- bounce via DRAM)
  - c supertile SWDGE cast-loaded straight to SBUF bf16
  - 4 gates = 4 PSUM 512-slices per 128-row subtile; per gate 1 bias matmul
    (ones-row trick) + 8 accumulating K=128 bf16 matmuls
  - ACT applies sigmoid/tanh straight out of PSUM into bf16 SBUF
  - DVE elementwise tail on whole supertiles; 2-class softmax folded to
    sigmoid(+/-d), d = h_new . (W_out[:,0]-W_out[:,1]) + (b_out[0]-b_out[1])
    via tensor_tensor_reduce
  - SWDGE cast-stores bf16 -> f32
"""
import sys

for _p in ("/root/.axon_site", "/root/.axon_site/_ro/trn_rl_repo",
           "/root/.axon_site/_ro/pypackages", "/opt/trn_rl_repo"):
    if _p not in sys.path:
        sys.path.append(_p)

import numpy as np
import ml_dtypes

import concourse.bacc as bacc
import concourse.tile as tile
from concourse import mybir
from concourse.bass_utils import run_bass_kernel_spmd

P = 128
H = 512
KC = H // P          # 4 feature chunks of 128
NCORES = 8
B = 32768
B_LOCAL = B // NCORES
NGATES = 4           # f, i, o, c_hat
GW = NGATES * H      # 2048 = concatenated gate width
ST = 512             # supertile batch rows
SUB = ST // P        # 4 subtiles per supertile

BF16 = mybir.dt.bfloat16
F32 = mybir.dt.float32
Sigmoid = mybir.ActivationFunctionType.Sigmoid
Tanh = mybir.ActivationFunctionType.Tanh


def build_program(b_local=B_LOCAL, num_devices=NCORES, repeat=1):
    nst = b_local // ST
    nc = bacc.Bacc("TRN2", target_bir_lowering=False, debug=False,
                   num_devices=num_devices)

    x_d = nc.dram_tensor("input_vector", (b_local, H), F32, kind="ExternalInput").ap()
    h_d = nc.dram_tensor("hidden_state", (b_local, H), F32, kind="ExternalInput").ap()
    c_d = nc.dram_tensor("cell_state", (b_local, H), F32, kind="ExternalInput").ap()
    # Wx[p, k, g*512+n] = U_g[k*128+p, n]; same for Wh/V_g. bf16, host-arranged.
    wx_d = nc.dram_tensor("Wx", (P, KC, GW), BF16, kind="ExternalInput").ap()
    wh_d = nc.dram_tensor("Wh", (P, KC, GW), BF16, kind="ExternalInput").ap()
    bias_d = nc.dram_tensor("bias_cat", (1, GW), BF16, kind="ExternalInput").ap()
    wd_d = nc.dram_tensor("w_diff", (1, H), BF16, kind="ExternalInput").ap()
    bd_d = nc.dram_tensor("b_diff", (1, 2), F32, kind="ExternalInput").ap()  # [+bd, -bd]

    pl_d = nc.dram_tensor("pre_label", (b_local, 2), F32, kind="ExternalOutput").ap()
    hn_d = nc.dram_tensor("h_new", (b_local, H), F32, kind="ExternalOutput").ap()
    cn_d = nc.dram_tensor("c_new", (b_local, H), F32, kind="ExternalOutput").ap()

    with tile.TileContext(nc) as tc:
        with tc.tile_pool(name="const", bufs=1) as const, \
             tc.tile_pool(name="work", bufs=2) as work, \
             tc.tile_pool(name="scr", bufs=3, space="DRAM") as scr, \
             tc.tile_pool(name="psum", bufs=2, space="PSUM") as psp:

            wx_sb = const.tile([P, KC, GW], BF16)
            nc.sync.dma_start(out=wx_sb, in_=wx_d)
            wh_sb = const.tile([P, KC, GW], BF16)
            nc.sync.dma_start(out=wh_sb, in_=wh_d)
            bias_sb = const.tile([1, GW], BF16)
            nc.sync.dma_start(out=bias_sb, in_=bias_d)
            ones = const.tile([1, P], BF16)
            nc.vector.memset(ones, 1.0)
            wd_sb = const.tile([P, H], BF16)
            nc.gpsimd.dma_start(out=wd_sb, in_=wd_d.to_broadcast((P, H)))
            bd_sb = const.tile([P, 2], F32)
            nc.gpsimd.dma_start(out=bd_sb, in_=bd_d.to_broadcast((P, 2)))
            p_all = const.tile([P, b_local // P, 2], F32)

            for st in range(nst * repeat):
                st = st % nst
                rows = slice(st * ST, (st + 1) * ST)

                x_scr = scr.tile([ST, H], BF16, tag="x_scr")
                nc.gpsimd.dma_start(out=x_scr, in_=x_d[rows, :])
                h_scr = scr.tile([ST, H], BF16, tag="h_scr")
                nc.gpsimd.dma_start(out=h_scr, in_=h_d[rows, :])

                xT = work.tile([P, KC, ST], BF16, tag="xT")
                nc.sync.dma_start(out=xT, in_=x_scr, transpose=True)
                hT = work.tile([P, KC, ST], BF16, tag="hT")
                nc.sync.dma_start(out=hT, in_=h_scr, transpose=True)

                c_bf = work.tile([P, SUB, H], BF16, tag="c_bf")
                nc.gpsimd.dma_start(
                    out=c_bf, in_=c_d[rows, :].rearrange("(s p) f -> p s f", p=P))

                gf = work.tile([P, SUB, H], BF16, tag="gf")
                gi = work.tile([P, SUB, H], BF16, tag="gi")
                go = work.tile([P, SUB, H], BF16, tag="go")
                gc = work.tile([P, SUB, H], BF16, tag="gc")
                gtiles = (gf, gi, go, gc)

                for s in range(SUB):
                    bs = slice(s * P, (s + 1) * P)
                    psum = psp.tile([P, GW], F32, tag="psum")
                    for g in range(NGATES):
                        gs = slice(g * H, (g + 1) * H)
                        nc.tensor.matmul(psum[:, gs], ones, bias_sb[:, gs],
                                         start=True, stop=False)
                    for k in range(KC):
                        for g in range(NGATES):
                            gs = slice(g * H, (g + 1) * H)
                            nc.tensor.matmul(psum[:, gs], xT[:, k, bs],
                                             wx_sb[:, k, gs],
                                             start=False, stop=False)
                    for k in range(KC):
                        for g in range(NGATES):
                            gs = slice(g * H, (g + 1) * H)
                            nc.tensor.matmul(psum[:, gs], hT[:, k, bs],
                                             wh_sb[:, k, gs],
                                             start=False, stop=(k == KC - 1))
                    for g in range(NGATES):
                        gs = slice(g * H, (g + 1) * H)
                        nc.scalar.activation(gtiles[g][:, s, :], psum[:, gs],
                                             Tanh if g == 3 else Sigmoid)

                flat = lambda t: t.rearrange("p s f -> p (s f)")
                t1 = work.tile([P, SUB * H], BF16, tag="t1")
                nc.vector.tensor_tensor(t1, flat(gf), flat(c_bf),
                                        mybir.AluOpType.mult)
                t2 = work.tile([P, SUB * H], BF16, tag="t2")
                nc.vector.tensor_tensor(t2, flat(gi), flat(gc),
                                        mybir.AluOpType.mult)
                cn = work.tile([P, SUB, H], BF16, tag="cn")
                nc.vector.tensor_tensor(flat(cn), t1, t2, mybir.AluOpType.add)
                tt = work.tile([P, SUB * H], BF16, tag="tt")
                nc.scalar.activation(tt, flat(cn), Tanh)
                hn = work.tile([P, SUB, H], BF16, tag="hn")
                nc.vector.tensor_tensor(flat(hn), flat(go), tt,
                                        mybir.AluOpType.mult)

                nc.gpsimd.dma_start(
                    out=cn_d[rows, :].rearrange("(s p) f -> p s f", p=P), in_=cn)
                nc.gpsimd.dma_start(
                    out=hn_d[rows, :].rearrange("(s p) f -> p s f", p=P), in_=hn)

                for s in range(SUB):
                    prod = work.tile([P, H], BF16, tag="prod")
                    nc.vector.tensor_tensor(prod, hn[:, s, :], wd_sb,
                                            mybir.AluOpType.mult)
                    d_sb = work.tile([P, 1], F32, tag="d_sb")
                    nc.vector.tensor_reduce(d_sb, prod, axis=mybir.AxisListType.X,
                                            op=mybir.AluOpType.add)
                    t_idx = st * SUB + s
                    nc.scalar.activation(p_all[:, t_idx, 0:1], d_sb, Sigmoid,
                                         bias=bd_sb[:, 0:1])
                    nc.scalar.activation(p_all[:, t_idx, 1:2], d_sb, Sigmoid,
                                         bias=bd_sb[:, 1:2], scale=-1.0)

            nc.sync.dma_start(
                out=pl_d.rearrange("(t p) c -> p t c", p=P), in_=p_all)

    nc.compile()
    return nc


_CACHE = {}


def _get_program():
    key = (B_LOCAL, NCORES)
    if key not in _CACHE:
        _CACHE[key] = build_program()
    return _CACHE[key]


def _prep_weights(U_f, V_f, b_f, U_i, V_i, b_i, U_o, V_o, b_o, U_c, V_c, b_c,
                  W_out, b_out):
    bf = ml_dtypes.bfloat16
    # [128, 4, 2048]: Wx[p, k, g*512:+512] = U_g[k*128+p, :]
    def arrange(*ms):
        cat = np.concatenate([np.asarray(m, np.float32).reshape(KC, P, H)
                              .transpose(1, 0, 2)[:, :, None, :] for m in ms], axis=2)
        return np.ascontiguousarray(cat.reshape(P, KC, GW)).astype(bf)

    wx = arrange(U_f, U_i, U_o, U_c)
    wh = arrange(V_f, V_i, V_o, V_c)
    bias = np.concatenate([np.asarray(b, np.float32) for b in (b_f, b_i, b_o, b_c)]
                          ).reshape(1, GW).astype(bf)
    W_out = np.asarray(W_out, np.float32)
    b_out = np.asarray(b_out, np.float32)
    wd = (W_out[:, 0] - W_out[:, 1]).reshape(1, H).astype(bf)
    bdv = float(b_out[0]) - float(b_out[1])
    bd = np.array([[bdv, -bdv]], np.float32)
    return wx, wh, bias, wd, bd


def make_in_maps(inputs):
    wx, wh, bias, wd, bd = _prep_weights(
        inputs["U_f"], inputs["V_f"], inputs["b_f"],
        inputs["U_i"], inputs["V_i"], inputs["b_i"],
        inputs["U_o"], inputs["V_o"], inputs["b_o"],
        inputs["U_c"], inputs["V_c"], inputs["b_c"],
        inputs["W_out"], inputs["b_out"])
    x = np.ascontiguousarray(np.asarray(inputs["input_vector"], np.float32))
    h = np.ascontiguousarray(np.asarray(inputs["hidden_state"], np.float32))
    c = np.ascontiguousarray(np.asarray(inputs["cell_state"], np.float32))
    in_maps = []
    for i in range(NCORES):
        rows = slice(i * B_LOCAL, (i + 1) * B_LOCAL)
        in_maps.append({
            "input_vector": x[rows], "hidden_state": h[rows],
            "cell_state": c[rows],
            "Wx": wx, "Wh": wh, "bias_cat": bias, "w_diff": wd, "b_diff": bd,
        })
    return in_maps


def kernel(**inputs):
    nc = _get_program()
    in_maps = make_in_maps(inputs)
    res = run_bass_kernel_spmd(nc, in_maps, core_ids=list(range(NCORES)))
    pre_label = np.concatenate([r["pre_label"] for r in res.results], axis=0)
    h_new = np.concatenate([r["h_new"] for r in res.results], axis=0)
    c_new = np.concatenate([r["c_new"] for r in res.results], axis=0)
    return pre_label, h_new, c_new


# revision 15
# speedup vs baseline: 227.5098x; 1.1026x over previous
"""Trainium2 Bass kernel for a single-step LSTM cell (nn_BasicLSTM).

reference math (B=32768, IN=H=512, NCLS=2):
    f = sigmoid(x @ U_f + h @ V_f + b_f)      i = sigmoid(x @ U_i + h @ V_i + b_i)
    o = sigmoid(x @ U_o + h @ V_o + b_o)      c_hat = tanh(x @ U_c + h @ V_c + b_c)
    c_new = f * c + i * c_hat                 h_new = o * tanh(c_new)
    pre_label = softmax(h_new @ W_out + b_out, axis=1)   # 2 classes

Strategy: pure data parallel over batch across 8 NeuronCores (4096 rows each).
Per core, 8 supertiles of 512 batch rows (each = 4 matmul subtiles of 128):
  - SWDGE cast-DMA x/h supertile f32 -> bf16 DRAM scratch, then one xbar
    DMA-transpose load per tensor into [K=feature, M=batch] chunk layout
    (SBUF->SBUF xbar transpose is fatal on this HW -- bounce via DRAM)
  - c supertile SWDGE cast-loaded straight to SBUF bf16
  - 4 gates = 4 PSUM 512-slices per 128-row subtile; per gate 1 bias matmul
    (ones-row trick) + 8 accumulating K=128 bf16 matmuls
  - ACT applies sigmoid/tanh straight out of PSUM into bf16 SBUF
  - DVE elementwise tail on whole supertiles; 2-class softmax folded to
    sigmoid(+/-d), d = h_new . (W_out[:,0]-W_out[:,1]) + (b_out[0]-b_out[1])
    via tensor_tensor_reduce
  - SWDGE cast-stores bf16 -> f32
"""
import sys

for _p in ("/root/.axon_site", "/root/.axon_site/_ro/trn_rl_repo",
           "/root/.axon_site/_ro/pypackages", "/opt/trn_rl_repo"):
    if _p not in sys.path:
        sys.path.append(_p)

import numpy as np
import ml_dtypes

import concourse.bacc as bacc
import concourse.tile as tile
from concourse import mybir
from concourse.bass_utils import run_bass_kernel_spmd

P = 128
H = 512
KC = H // P          # 4 feature chunks of 128
NCORES = 8
B = 32768
B_LOCAL = B // NCORES
NGATES = 4           # f, i, o, c_hat
GW = NGATES * H      # 2048 = concatenated gate width
ST = 512             # supertile batch rows
SUB = ST // P        # 4 subtiles per supertile

BF16 = mybir.dt.bfloat16
F32 = mybir.dt.float32
Sigmoid = mybir.ActivationFunctionType.Sigmoid
Tanh = mybir.ActivationFunctionType.Tanh


def build_program(b_local=B_LOCAL, num_devices=NCORES, repeat=1):
    nst = b_local // ST
    nc = bacc.Bacc("TRN2", target_bir_lowering=False, debug=False,
                   num_devices=num_devices, num_swdge_queues=4)

    x_d = nc.dram_tensor("input_vector", (b_local, H), F32, kind="ExternalInput").ap()
    h_d = nc.dram_tensor("hidden_state", (b_local, H), F32, kind="ExternalInput").ap()
    c_d = nc.dram_tensor("cell_state", (b_local, H), F32, kind="ExternalInput").ap()
    # Wx[p, k, g*512+n] = U_g[k*128+p, n]; same for Wh/V_g. bf16, host-arranged.
    wx_d = nc.dram_tensor("Wx", (P, KC, GW), BF16, kind="ExternalInput").ap()
    wh_d = nc.dram_tensor("Wh", (P, KC, GW), BF16, kind="ExternalInput").ap()
    bias_d = nc.dram_tensor("bias_cat", (1, GW), BF16, kind="ExternalInput").ap()
    wd_d = nc.dram_tensor("w_diff", (1, H), BF16, kind="ExternalInput").ap()
    bd_d = nc.dram_tensor("b_diff", (1, 2), F32, kind="ExternalInput").ap()  # [+bd, -bd]

    pl_d = nc.dram_tensor("pre_label", (b_local, 2), F32, kind="ExternalOutput").ap()
    hn_d = nc.dram_tensor("h_new", (b_local, H), F32, kind="ExternalOutput").ap()
    cn_d = nc.dram_tensor("c_new", (b_local, H), F32, kind="ExternalOutput").ap()

    with tile.TileContext(nc) as tc:
        with tc.tile_pool(name="const", bufs=1) as const, \
             tc.tile_pool(name="work", bufs=2) as work, \
             tc.tile_pool(name="scr", bufs=3, space="DRAM") as scr, \
             tc.tile_pool(name="psum", bufs=2, space="PSUM") as psp:

            wx_sb = const.tile([P, KC, GW], BF16)
            nc.scalar.dma_start(out=wx_sb, in_=wx_d)
            wh_sb = const.tile([P, KC, GW], BF16)
            nc.scalar.dma_start(out=wh_sb, in_=wh_d)
            bias_sb = const.tile([P, GW], BF16)
            nc.gpsimd.dma_start(out=bias_sb, in_=bias_d.to_broadcast((P, GW)))
            wd_sb = const.tile([P, H], BF16)
            nc.gpsimd.dma_start(out=wd_sb, in_=wd_d.to_broadcast((P, H)))
            bd_sb = const.tile([P, 2], F32)
            nc.gpsimd.dma_start(out=bd_sb, in_=bd_d.to_broadcast((P, 2)))
            for st in range(nst * repeat):
                st = st % nst
                rows = slice(st * ST, (st + 1) * ST)

                xh_scr = scr.tile([2 * ST, H], BF16, tag="xh_scr")
                nc.gpsimd.dma_start(out=xh_scr[0:ST, :], in_=x_d[rows, :])
                nc.gpsimd.dma_start(out=xh_scr[ST:2 * ST, :], in_=h_d[rows, :])

                xhT = work.tile([P, KC, 2 * ST], BF16, tag="xhT")
                nc.sync.dma_start(out=xhT, in_=xh_scr, transpose=True)
                xT = xhT[:, :, 0:ST]
                hT = xhT[:, :, ST:2 * ST]

                c_bf = work.tile([P, SUB, H], BF16, tag="c_bf")
                nc.gpsimd.dma_start(
                    out=c_bf, in_=c_d[rows, :].rearrange("(s p) f -> p s f", p=P))

                gf = work.tile([P, SUB, H], BF16, tag="gf")
                gi = work.tile([P, SUB, H], BF16, tag="gi")
                go = work.tile([P, SUB, H], BF16, tag="go")
                gc = work.tile([P, SUB, H], BF16, tag="gc")
                gtiles = (gf, gi, go, gc)

                for s in range(SUB):
                    bs = slice(s * P, (s + 1) * P)
                    psum = psp.tile([P, GW], F32, tag="psum")
                    for k in range(KC):
                        for g in range(NGATES):
                            gs = slice(g * H, (g + 1) * H)
                            nc.tensor.matmul(psum[:, gs], xT[:, k, bs],
                                             wx_sb[:, k, gs],
                                             start=(k == 0), stop=False)
                    for k in range(KC):
                        for g in range(NGATES):
                            gs = slice(g * H, (g + 1) * H)
                            nc.tensor.matmul(psum[:, gs], hT[:, k, bs],
                                             wh_sb[:, k, gs],
                                             start=False, stop=(k == KC - 1))
                    nc.vector.tensor_tensor(psum, psum, bias_sb,
                                            mybir.AluOpType.add)
                    for g in range(NGATES):
                        gs = slice(g * H, (g + 1) * H)
                        nc.scalar.activation(gtiles[g][:, s, :], psum[:, gs],
                                             Tanh if g == 3 else Sigmoid)

                flat = lambda t: t.rearrange("p s f -> p (s f)")
                t1 = work.tile([P, SUB * H], BF16, tag="t1")
                nc.vector.tensor_tensor(t1, flat(gf), flat(c_bf),
                                        mybir.AluOpType.mult)
                t2 = work.tile([P, SUB * H], BF16, tag="t2")
                nc.vector.tensor_tensor(t2, flat(gi), flat(gc),
                                        mybir.AluOpType.mult)
                cn = work.tile([P, SUB, H], BF16, tag="cn")
                nc.vector.tensor_tensor(flat(cn), t1, t2, mybir.AluOpType.add)
                tt = work.tile([P, SUB * H], BF16, tag="tt")
                nc.scalar.activation(tt, flat(cn), Tanh)
                hn = work.tile([P, SUB, H], BF16, tag="hn")
                nc.vector.tensor_tensor(flat(hn), flat(go), tt,
                                        mybir.AluOpType.mult)

                nc.gpsimd.dma_start(
                    out=cn_d[rows, :].rearrange("(s p) f -> p s f", p=P), in_=cn)
                nc.gpsimd.dma_start(
                    out=hn_d[rows, :].rearrange("(s p) f -> p s f", p=P), in_=hn)

                p_st = work.tile([P, SUB, 2], F32, tag="p_st")
                for s in range(SUB):
                    prod = work.tile([P, H], BF16, tag="prod")
                    nc.vector.tensor_tensor(prod, hn[:, s, :], wd_sb,
                                            mybir.AluOpType.mult)
                    d_sb = work.tile([P, 1], F32, tag="d_sb")
                    nc.vector.tensor_reduce(d_sb, prod, axis=mybir.AxisListType.X,
                                            op=mybir.AluOpType.add)
                    nc.scalar.activation(p_st[:, s, 0:1], d_sb, Sigmoid,
                                         bias=bd_sb[:, 0:1])
                    nc.scalar.activation(p_st[:, s, 1:2], d_sb, Sigmoid,
                                         bias=bd_sb[:, 1:2], scale=-1.0)
                nc.scalar.dma_start(
                    out=pl_d[rows, :].rearrange("(s p) c -> p s c", p=P),
                    in_=p_st)

    nc.compile()
    return nc


_CACHE = {}


def _get_program():
    key = (B_LOCAL, NCORES)
    if key not in _CACHE:
        _CACHE[key] = build_program()
    return _CACHE[key]


def _prep_weights(U_f, V_f, b_f, U_i, V_i, b_i, U_o, V_o, b_o, U_c, V_c, b_c,
                  W_out, b_out):
    bf = ml_dtypes.bfloat16
    # [128, 4, 2048]: Wx[p, k, g*512:+512] = U_g[k*128+p, :]
    def arrange(*ms):
        cat = np.concatenate([np.asarray(m, np.float32).reshape(KC, P, H)
                              .transpose(1, 0, 2)[:, :, None, :] for m in ms], axis=2)
        return np.ascontiguousarray(cat.reshape(P, KC, GW)).astype(bf)

    wx = arrange(U_f, U_i, U_o, U_c)
    wh = arrange(V_f, V_i, V_o, V_c)
    bias = np.concatenate([np.asarray(b, np.float32) for b in (b_f, b_i, b_o, b_c)]
                          ).reshape(1, GW).astype(bf)
    W_out = np.asarray(W_out, np.float32)
    b_out = np.asarray(b_out, np.float32)
    wd = (W_out[:, 0] - W_out[:, 1]).reshape(1, H).astype(bf)
    bdv = float(b_out[0]) - float(b_out[1])
    bd = np.array([[bdv, -bdv]], np.float32)
    return wx, wh, bias, wd, bd


def make_in_maps(inputs):
    wx, wh, bias, wd, bd = _prep_weights(
        inputs["U_f"], inputs["V_f"], inputs["b_f"],
        inputs["U_i"], inputs["V_i"], inputs["b_i"],
        inputs["U_o"], inputs["V_o"], inputs["b_o"],
        inputs["U_c"], inputs["V_c"], inputs["b_c"],
        inputs["W_out"], inputs["b_out"])
    x = np.ascontiguousarray(np.asarray(inputs["input_vector"], np.float32))
    h = np.ascontiguousarray(np.asarray(inputs["hidden_state"], np.float32))
    c = np.ascontiguousarray(np.asarray(inputs["cell_state"], np.float32))
    in_maps = []
    for i in range(NCORES):
        rows = slice(i * B_LOCAL, (i + 1) * B_LOCAL)
        in_maps.append({
            "input_vector": x[rows], "hidden_state": h[rows],
            "cell_state": c[rows],
            "Wx": wx, "Wh": wh, "bias_cat": bias, "w_diff": wd, "b_diff": bd,
        })
    return in_maps


def kernel(**inputs):
    nc = _get_program()
    in_maps = make_in_maps(inputs)
    res = run_bass_kernel_spmd(nc, in_maps, core_ids=list(range(NCORES)))
    pre_label = np.concatenate([r["pre_label"] for r in res.results], axis=0)
    h_new = np.concatenate([r["h_new"] for r in res.results], axis=0)
    c_new = np.concatenate([r["c_new"] for r in res.results], axis=0)
    return pre_label, h_new, c_new


# revision 16
# speedup vs baseline: 238.9243x; 1.0502x over previous
"""Trainium2 Bass kernel for a single-step LSTM cell (nn_BasicLSTM).

reference math (B=32768, IN=H=512, NCLS=2):
    f = sigmoid(x @ U_f + h @ V_f + b_f)      i = sigmoid(x @ U_i + h @ V_i + b_i)
    o = sigmoid(x @ U_o + h @ V_o + b_o)      c_hat = tanh(x @ U_c + h @ V_c + b_c)
    c_new = f * c + i * c_hat                 h_new = o * tanh(c_new)
    pre_label = softmax(h_new @ W_out + b_out, axis=1)   # 2 classes

Strategy: pure data parallel over batch across 8 NeuronCores (4096 rows each).
Per core, 8 supertiles of 512 batch rows (each = 4 matmul subtiles of 128):
  - SWDGE cast-DMA x/h supertile f32 -> bf16 DRAM scratch, then one xbar
    DMA-transpose load per tensor into [K=feature, M=batch] chunk layout
    (SBUF->SBUF xbar transpose is fatal on this HW -- bounce via DRAM)
  - c supertile SWDGE cast-loaded straight to SBUF bf16
  - 4 gates = 4 PSUM 512-slices per 128-row subtile; per gate 1 bias matmul
    (ones-row trick) + 8 accumulating K=128 bf16 matmuls
  - ACT applies sigmoid/tanh straight out of PSUM into bf16 SBUF
  - DVE elementwise tail on whole supertiles; 2-class softmax folded to
    sigmoid(+/-d), d = h_new . (W_out[:,0]-W_out[:,1]) + (b_out[0]-b_out[1])
    via tensor_tensor_reduce
  - SWDGE cast-stores bf16 -> f32
"""
import sys

for _p in ("/root/.axon_site", "/root/.axon_site/_ro/trn_rl_repo",
           "/root/.axon_site/_ro/pypackages", "/opt/trn_rl_repo"):
    if _p not in sys.path:
        sys.path.append(_p)

import numpy as np
import ml_dtypes

import concourse.bacc as bacc
import concourse.tile as tile
from concourse import mybir
from concourse.bass_utils import run_bass_kernel_spmd

P = 128
H = 512
KC = H // P          # 4 feature chunks of 128
NCORES = 8
B = 32768
B_LOCAL = B // NCORES
NGATES = 4           # f, i, o, c_hat
GW = NGATES * H      # 2048 = concatenated gate width
ST = 512             # supertile batch rows
SUB = ST // P        # 4 subtiles per supertile

BF16 = mybir.dt.bfloat16
F32 = mybir.dt.float32
Sigmoid = mybir.ActivationFunctionType.Sigmoid
Tanh = mybir.ActivationFunctionType.Tanh


def build_program(b_local=B_LOCAL, num_devices=NCORES, repeat=1,
                  work_bufs=2, scr_bufs=3):
    nst = b_local // ST
    nc = bacc.Bacc("TRN2", target_bir_lowering=False, debug=False,
                   num_devices=num_devices, num_swdge_queues=4)

    x_d = nc.dram_tensor("input_vector", (b_local, H), F32, kind="ExternalInput").ap()
    h_d = nc.dram_tensor("hidden_state", (b_local, H), F32, kind="ExternalInput").ap()
    c_d = nc.dram_tensor("cell_state", (b_local, H), F32, kind="ExternalInput").ap()
    # Wx[p, k, g*512+n] = U_g[k*128+p, n]; same for Wh/V_g. bf16, host-arranged.
    wx_d = nc.dram_tensor("Wx", (P, KC, GW), BF16, kind="ExternalInput").ap()
    wh_d = nc.dram_tensor("Wh", (P, KC, GW), BF16, kind="ExternalInput").ap()
    bias_d = nc.dram_tensor("bias_cat", (1, GW), BF16, kind="ExternalInput").ap()
    wd_d = nc.dram_tensor("w_diff", (1, H), BF16, kind="ExternalInput").ap()
    bd_d = nc.dram_tensor("b_diff", (1, 2), F32, kind="ExternalInput").ap()  # [+bd, -bd]

    pl_d = nc.dram_tensor("pre_label", (b_local, 2), F32, kind="ExternalOutput").ap()
    hn_d = nc.dram_tensor("h_new", (b_local, H), F32, kind="ExternalOutput").ap()
    cn_d = nc.dram_tensor("c_new", (b_local, H), F32, kind="ExternalOutput").ap()

    with tile.TileContext(nc) as tc:
        with tc.tile_pool(name="const", bufs=1) as const, \
             tc.tile_pool(name="work", bufs=work_bufs) as work, \
             tc.tile_pool(name="scr", bufs=scr_bufs, space="DRAM") as scr, \
             tc.tile_pool(name="psum", bufs=2, space="PSUM") as psp:

            wx_sb = const.tile([P, KC, GW], BF16)
            nc.scalar.dma_start(out=wx_sb, in_=wx_d)
            wh_sb = const.tile([P, KC, GW], BF16)
            nc.scalar.dma_start(out=wh_sb, in_=wh_d)
            bias_sb = const.tile([P, GW], BF16)
            nc.gpsimd.dma_start(out=bias_sb, in_=bias_d.to_broadcast((P, GW)))
            wd_sb = const.tile([P, H], BF16)
            nc.gpsimd.dma_start(out=wd_sb, in_=wd_d.to_broadcast((P, H)))
            bd_sb = const.tile([P, 2], F32)
            nc.gpsimd.dma_start(out=bd_sb, in_=bd_d.to_broadcast((P, 2)))
            for st in range(nst * repeat):
                st = st % nst
                rows = slice(st * ST, (st + 1) * ST)

                xh_scr = scr.tile([2 * ST, H], BF16, tag="xh_scr")
                nc.gpsimd.dma_start(out=xh_scr[0:ST, :], in_=x_d[rows, :])
                nc.gpsimd.dma_start(out=xh_scr[ST:2 * ST, :], in_=h_d[rows, :])

                xhT = work.tile([P, KC, 2 * ST], BF16, tag="xhT")
                nc.sync.dma_start(out=xhT, in_=xh_scr, transpose=True)
                xT = xhT[:, :, 0:ST]
                hT = xhT[:, :, ST:2 * ST]

                c_bf = work.tile([P, SUB, H], BF16, tag="c_bf")
                nc.gpsimd.dma_start(
                    out=c_bf, in_=c_d[rows, :].rearrange("(s p) f -> p s f", p=P))

                gf = work.tile([P, SUB, H], BF16, tag="gf")
                gi = work.tile([P, SUB, H], BF16, tag="gi")
                go = work.tile([P, SUB, H], BF16, tag="go")
                gc = work.tile([P, SUB, H], BF16, tag="gc")
                gtiles = (gf, gi, go, gc)

                for s in range(SUB):
                    bs = slice(s * P, (s + 1) * P)
                    psum = psp.tile([P, GW], F32, tag="psum")
                    for k in range(KC):
                        for g in range(NGATES):
                            gs = slice(g * H, (g + 1) * H)
                            nc.tensor.matmul(psum[:, gs], xT[:, k, bs],
                                             wx_sb[:, k, gs],
                                             start=(k == 0), stop=False)
                    for k in range(KC):
                        for g in range(NGATES):
                            gs = slice(g * H, (g + 1) * H)
                            nc.tensor.matmul(psum[:, gs], hT[:, k, bs],
                                             wh_sb[:, k, gs],
                                             start=False, stop=(k == KC - 1))
                    nc.vector.tensor_tensor(psum, psum, bias_sb,
                                            mybir.AluOpType.add)
                    for g in range(NGATES):
                        gs = slice(g * H, (g + 1) * H)
                        nc.scalar.activation(gtiles[g][:, s, :], psum[:, gs],
                                             Tanh if g == 3 else Sigmoid)

                flat = lambda t: t.rearrange("p s f -> p (s f)")
                t1 = work.tile([P, SUB * H], BF16, tag="t1")
                nc.vector.tensor_tensor(t1, flat(gf), flat(c_bf),
                                        mybir.AluOpType.mult)
                t2 = work.tile([P, SUB * H], BF16, tag="t2")
                nc.vector.tensor_tensor(t2, flat(gi), flat(gc),
                                        mybir.AluOpType.mult)
                cn = work.tile([P, SUB, H], BF16, tag="cn")
                nc.vector.tensor_tensor(flat(cn), t1, t2, mybir.AluOpType.add)
                tt = work.tile([P, SUB * H], BF16, tag="tt")
                nc.scalar.activation(tt, flat(cn), Tanh)
                hn = work.tile([P, SUB, H], BF16, tag="hn")
                nc.vector.tensor_tensor(flat(hn), flat(go), tt,
                                        mybir.AluOpType.mult)

                nc.gpsimd.dma_start(
                    out=cn_d[rows, :].rearrange("(s p) f -> p s f", p=P), in_=cn)
                nc.gpsimd.dma_start(
                    out=hn_d[rows, :].rearrange("(s p) f -> p s f", p=P), in_=hn)

                p_st = work.tile([P, SUB, 2], F32, tag="p_st")
                for s in range(SUB):
                    prod = work.tile([P, H], BF16, tag="prod")
                    nc.vector.tensor_tensor(prod, hn[:, s, :], wd_sb,
                                            mybir.AluOpType.mult)
                    d_sb = work.tile([P, 1], F32, tag="d_sb")
                    nc.vector.tensor_reduce(d_sb, prod, axis=mybir.AxisListType.X,
                                            op=mybir.AluOpType.add)
                    nc.scalar.activation(p_st[:, s, 0:1], d_sb, Sigmoid,
                                         bias=bd_sb[:, 0:1])
                    nc.scalar.activation(p_st[:, s, 1:2], d_sb, Sigmoid,
                                         bias=bd_sb[:, 1:2], scale=-1.0)
                nc.scalar.dma_start(
                    out=pl_d[rows, :].rearrange("(s p) c -> p s c", p=P),
                    in_=p_st)

    nc.compile()
    return nc


_CACHE = {}


def _get_program():
    key = (B_LOCAL, NCORES)
    if key not in _CACHE:
        _CACHE[key] = build_program()
    return _CACHE[key]


def _prep_weights(U_f, V_f, b_f, U_i, V_i, b_i, U_o, V_o, b_o, U_c, V_c, b_c,
                  W_out, b_out):
    bf = ml_dtypes.bfloat16
    # [128, 4, 2048]: Wx[p, k, g*512:+512] = U_g[k*128+p, :]
    def arrange(*ms):
        cat = np.concatenate([np.asarray(m, np.float32).reshape(KC, P, H)
                              .transpose(1, 0, 2)[:, :, None, :] for m in ms], axis=2)
        return np.ascontiguousarray(cat.reshape(P, KC, GW)).astype(bf)

    wx = arrange(U_f, U_i, U_o, U_c)
    wh = arrange(V_f, V_i, V_o, V_c)
    bias = np.concatenate([np.asarray(b, np.float32) for b in (b_f, b_i, b_o, b_c)]
                          ).reshape(1, GW).astype(bf)
    W_out = np.asarray(W_out, np.float32)
    b_out = np.asarray(b_out, np.float32)
    wd = (W_out[:, 0] - W_out[:, 1]).reshape(1, H).astype(bf)
    bdv = float(b_out[0]) - float(b_out[1])
    bd = np.array([[bdv, -bdv]], np.float32)
    return wx, wh, bias, wd, bd


def make_in_maps(inputs):
    wx, wh, bias, wd, bd = _prep_weights(
        inputs["U_f"], inputs["V_f"], inputs["b_f"],
        inputs["U_i"], inputs["V_i"], inputs["b_i"],
        inputs["U_o"], inputs["V_o"], inputs["b_o"],
        inputs["U_c"], inputs["V_c"], inputs["b_c"],
        inputs["W_out"], inputs["b_out"])
    x = np.ascontiguousarray(np.asarray(inputs["input_vector"], np.float32))
    h = np.ascontiguousarray(np.asarray(inputs["hidden_state"], np.float32))
    c = np.ascontiguousarray(np.asarray(inputs["cell_state"], np.float32))
    in_maps = []
    for i in range(NCORES):
        rows = slice(i * B_LOCAL, (i + 1) * B_LOCAL)
        in_maps.append({
            "input_vector": x[rows], "hidden_state": h[rows],
            "cell_state": c[rows],
            "Wx": wx, "Wh": wh, "bias_cat": bias, "w_diff": wd, "b_diff": bd,
        })
    return in_maps


def kernel(**inputs):
    nc = _get_program()
    in_maps = make_in_maps(inputs)
    res = run_bass_kernel_spmd(nc, in_maps, core_ids=list(range(NCORES)))
    pre_label = np.concatenate([r["pre_label"] for r in res.results], axis=0)
    h_new = np.concatenate([r["h_new"] for r in res.results], axis=0)
    c_new = np.concatenate([r["c_new"] for r in res.results], axis=0)
    return pre_label, h_new, c_new
